# revision 23
# baseline (speedup 1.0000x reference)
"""Trainium2 Bass kernel for single-head attention (B=4, S=2048, D=H=1024).

Sharding: 8 cores = 4 batches x 2 query-halves. Each core computes the
attention output for 1024 query rows of one batch. The sequence axis is
rotated per-core on the host so the core's query rows always occupy
positions 0..1023 (softmax and the sum over keys are permutation-
invariant, so key order doesn't matter as long as xT/xO agree).

Algebraic restructuring (weights-only folding, done host-side in fp32):
  M  = Wq @ Wk^T          [D,D]   scores = x_q M x_k^T (+ r.x_k)
  r  = Wk @ bq            [D]     per-key score bias; the per-query terms
                                  (x_q Wq).bk + bq.bk are softmax-invariant
                                  and dropped
  W2 = Wv @ Wo            [D,D]   z = (En @ x) @ W2 + c
  c  = Wo^T @ bv + bo     [D]     (uses sum_k En = 1)
This removes the K and V projections entirely: the score and output GEMMs
contract directly against x. Per-core matmul work drops from 10 x 2^30 to
6 x 2^30 MACs (768 N=512 matmuls).

Per-core pipeline (all matmuls fp16 operands, fp32 PSUM):
  A1: TT[j,q] = M^T xq + r   (SBUF-resident, 128 MMs; stationary M-block
      reused across the two 512-query chunks)
  B:  per q-tile: S = TT^T xT (256 MMs; stationary TT-block reused across
      the four 512-key chunks) -> rowmax -> Exp(bias=-max, accum_out=den)
      -> En = E/den fp16 -> DRAM (EnD). ETc transpose-loads for each
      512-query chunk are issued as soon as its 4 q-tiles are stored.
  C:  y'T = x^T ETc per chunk (256 MMs), then zT = W2^T y'T + c for both
      chunks (128 MMs; stationary W2-block reused across chunks) -> DRAM.
      Host transposes zT back to [B,S,D].
"""

import sys

import numpy as np

for _p in ("/opt/trn_rl_repo",):
    if _p not in sys.path:
        sys.path.insert(0, _p)

import concourse.bass as bass
import concourse.mybir as mybir
import concourse.tile as tile
from concourse.bass_utils import run_bass_kernel_spmd


def _install_profile_shims():
    """This image's `antenv` lacks `axon_hooks`, which run_bass_kernel_spmd
    imports for trace=True under axon; libaxon_pjrt.so has the NTFF symbols.
    Register a stand-in module wired to the ctypes hook, and neuter the
    artifact upload (zero-egress container)."""
    import types

    try:
        import antenv.axon_hooks  # noqa: F401
    except ImportError:
        hook = None
        try:
            import trn_agent_boot.trn_boot as _tb

            hook = _tb._ntff_profile_via_ctypes("/opt/axon/libaxon_pjrt.so")
        except Exception:
            hook = None
        import antenv

        m = types.ModuleType("antenv.axon_hooks")
        m.get_axon_ntff_profile_hook = lambda: hook
        m.set_axon_ntff_profile_hook = lambda h: None
        sys.modules["antenv.axon_hooks"] = m
        antenv.axon_hooks = m

    import concourse.bass_utils as _bu

    _bu.upload_artifacts = lambda tmpdir: tmpdir


_install_profile_shims()

B, S, D, H = 4, 2048, 1024, 1024
P = 128
NQ = 1024  # query rows per core
D_T, S_T, Q_T = D // P, S // P, NQ // P
KC, QC = S // 512, NQ // 512

F32 = mybir.dt.float32
F16 = mybir.dt.float16
Ident = mybir.ActivationFunctionType.Identity


def _split_multi_waits(nc, max_waits=1):
    """This container's walrus rejects >1 sync wait on NO_STRUCT opcodes
    (Drain/NoOp). Move extra waits onto dedicated single-wait NoOps inserted
    right before the offending instruction on the same engine."""
    for f in nc.m.functions:
        for bb in f.blocks:
            insts = bb.instructions
            i = 0
            while i < len(insts):
                ins = insts[i]
                si = ins.sync_info
                if si is not None and si.on_wait and len(si.on_wait) > max_waits:
                    waits = list(si.on_wait)
                    si.on_wait = waits[:max_waits]
                    ins.sync_info = si
                    for j, w in enumerate(waits[max_waits:]):
                        nop = mybir.InstNoOp(
                            name=f"{ins.name}-waitsplit-{j}",
                            engine=ins.engine,
                            bass_nofuse=True,
                            sync_info=mybir.SyncInfo(on_wait=[w], on_update=[]),
                        )
                        insts.insert(i, nop)
                        i += 1
                i += 1
            bb.instructions = insts


def _build(split_waits=True):
    nc = bass.Bass()

    def din(name, shape, dt=F16):
        return nc.declare_dram_parameter(name, shape, dt, isOutput=False)

    xTd = din("xT", [D, S])      # x[b]^T, seq rotated so queries at cols 0:NQ
    xOd = din("xO", [S, D])      # x[b], same rotation on rows
    Md = din("M", [D, D])        # Wq Wk^T fp16
    W2d = din("W2", [D, D])      # Wv Wo   fp16
    rd = din("r", [D], F32)      # Wk bq
    cd = din("c", [D], F32)      # Wo^T bv + bo
    zT = nc.declare_dram_parameter("zT", [D, NQ], F16, isOutput=True)

    with tile.TileContext(nc) as tc:
        with (
            tc.tile_pool(name="pers", bufs=1) as pers,
            tc.tile_pool(name="dram", bufs=1, space="DRAM") as dramp,
            tc.tile_pool(name="ps", bufs=8, space="PSUM") as psp,
        ):
            EnD = dramp.tile([NQ, S], F16, tag="EnD", name="EnD")

            # --- DMA priority order: first MM group needs M[0] + xq(xT[0]).
            # xT tiles are loaded in two 1024-col halves so the query half
            # lands first. All loads stay on the sync HWDGE queue: moving
            # some to the scalar queue measurably delayed them (late queue
            # start + contention with Act compute).
            Ms = [pers.tile([P, D], F16, tag=f"m{d}", name=f"m{d}")
                  for d in range(D_T)]
            xTs = [pers.tile([P, S], F16, tag=f"xt{d}", name=f"xt{d}")
                   for d in range(D_T)]
            # d=0 pair split in halves, ordered to match A1's qc-first
            # emission for d=0: the first 4 MMs (t0-3, qc0) need only
            # M0[:, 0:512] + xT0[:, 0:512]; qc1 needs xT0[:, 512:1024];
            # M0[:, 512:] is th=1 material (~26us in), so it loads last.
            nc.sync.dma_start(out=Ms[0][:, 0:512], in_=Md[0:P, 0:512])
            nc.sync.dma_start(out=xTs[0][:, 0:512], in_=xTd[0:P, 0:512])
            nc.sync.dma_start(out=xTs[0][:, 512:NQ], in_=xTd[0:P, 512:NQ])
            nc.sync.dma_start(out=Ms[0][:, 512:D], in_=Md[0:P, 512:D])
            for d in range(1, D_T):
                r_ = slice(d * P, (d + 1) * P)
                nc.sync.dma_start(out=Ms[d][:], in_=Md[r_, :])
                nc.sync.dma_start(out=xTs[d][:, 0:NQ], in_=xTd[r_, 0:NQ])
            bias_r = pers.tile([P, D_T], F32, tag="br", name="br")
            bias_c = pers.tile([P, D_T], F32, tag="bc", name="bc")
            nc.sync.dma_start(out=bias_r[:], in_=rd.rearrange("(t p) -> p t", p=P))
            nc.sync.dma_start(out=bias_c[:], in_=cd.rearrange("(t p) -> p t", p=P))
            for d in range(D_T):
                r_ = slice(d * P, (d + 1) * P)
                nc.sync.dma_start(out=xTs[d][:, NQ:S], in_=xTd[r_, NQ:S])
            xOs = []
            for s in range(S_T):
                r_ = slice(s * P, (s + 1) * P)
                ot = pers.tile([P, D], F16, tag=f"xo{s}", name=f"xo{s}")
                nc.sync.dma_start(out=ot[:], in_=xOd[r_, :])
                xOs.append(ot)
            W2s = []
            for d in range(D_T):
                r_ = slice(d * P, (d + 1) * P)
                wt = pers.tile([P, D], F16, tag=f"w2{d}", name=f"w2{d}")
                nc.sync.dma_start(out=wt[:], in_=W2d[r_, :])
                W2s.append(wt)

            # TT: [j, q] fp16, 8 j-tiles of [128, 1024]
            TT = [pers.tile([P, NQ], F16, tag=f"tt{t}", name=f"tt{t}")
                  for t in range(D_T)]

            # --- PE warmup: matmuls on a zeroed tile while input DMAs land,
            # so the HAM un-throttles before real work starts.
            warm = pers.tile([P, P], F16, tag="warm", name="warm")
            nc.vector.memset(warm[:], 0.0)
            wps = psp.tile([P, 512], F32, tag="ps", name="ps")
            for _ in range(32):
                nc.tensor.matmul(wps[:, 0:P], warm[:], warm[:],
                                 start=True, stop=True)

            # ---- A1: TT = M^T xq + r. d-outer; within d, the M-block
            # stationary operand is reused for both 512-query chunks.
            # t split into two halves so 2qc x 4t = 8 PSUM banks.
            for th in range(2):
                ts = range(th * 4, th * 4 + 4)
                psA = {(qc, t): psp.tile([P, 512], F32, tag="ps", name="ps")
                       for t in ts for qc in range(QC)}
                for d in range(D_T):
                    for qc in range(QC):
                        for t in ts:
                            nc.tensor.matmul(
                                psA[qc, t][:], Ms[d][:, t * P : (t + 1) * P],
                                xTs[d][:, qc * 512 : (qc + 1) * 512],
                                start=(d == 0), stop=(d == D_T - 1))
                for t in ts:
                    for qc in range(QC):
                        nc.scalar.activation(
                            TT[t][:, qc * 512 : (qc + 1) * 512],
                            psA[qc, t][:], Ident,
                            bias=bias_r[:, t : t + 1])

            # ---- B: scores + softmax -> EnD; ETc prefetch per 512-q chunk -
            ETc = [[None] * S_T for _ in range(QC)]
            with tc.tile_pool(name="pB", bufs=2) as pB:
                for qt in range(Q_T):
                    qsl = slice(qt * P, (qt + 1) * P)
                    Ssb = pB.tile([P, S], F32, tag="Ssb", name="Ssb")
                    pss = [psp.tile([P, 512], F32, tag="ps", name="ps")
                           for _ in range(KC)]
                    for t in range(D_T):
                        for kc in range(KC):
                            nc.tensor.matmul(
                                pss[kc][:], TT[t][:, qsl],
                                xTs[t][:, kc * 512 : (kc + 1) * 512],
                                start=(t == 0), stop=(t == D_T - 1))
                    for kc in range(KC):
                        nc.vector.tensor_copy(
                            Ssb[:, kc * 512 : (kc + 1) * 512], pss[kc][:])
                    nmx = pB.tile([P, 1], F32, tag="nmx", name="nmx")
                    nc.vector.reduce_max(nmx[:], Ssb[:],
                                         axis=mybir.AxisListType.X,
                                         negate=True)
                    En = pB.tile([P, S], F16, tag="En", name="En")
                    den = pB.tile([P, 1], F32, tag="den", name="den")
                    nc.scalar.activation(
                        En[:], Ssb[:], mybir.ActivationFunctionType.Exp,
                        bias=nmx[:], accum_out=den[:])
                    rec = pB.tile([P, 1], F32, tag="rec", name="rec")
                    nc.vector.reciprocal(rec[:], den[:])
                    Enn = pB.tile([P, S], F16, tag="Enn", name="Enn")
                    nc.scalar.mul(Enn[:], En[:], rec[:])
                    nc.sync.dma_start(out=EnD[qt * P : (qt + 1) * P, :],
                                      in_=Enn[:])
                    # Transposed ETc reads. One queue only: concurrent
                    # DMA_TRANSPOSE from two HWDGE queues corrupts data
                    # (shared XBAR), and finer-grained [128,128] transposes
                    # hang the exec unit. Chunk a's 16 triggers (~1.3us
                    # each) are spread in 4-batches across qt 3..6 so the
                    # qt4-7 Enn writes (which gate chunk b's transposes)
                    # are not starved behind a 20us trigger burst.
                    if qt == 3:
                        ETc[0] = [pers.tile([P, 512], F16, tag=f"et{s}",
                                            name=f"et{s}", bufs=2)
                                  for s in range(S_T)]
                    if 3 <= qt <= 6:
                        for s in range(4 * (qt - 3), 4 * (qt - 3) + 4):
                            nc.sync.dma_start(
                                out=ETc[0][s][:],
                                in_=EnD[0:512, s * P : (s + 1) * P],
                                transpose=True)
                    if qt == 7:
                        ETc[1] = [pers.tile([P, 512], F16, tag=f"et{s}",
                                            name=f"et{s}", bufs=2)
                                  for s in range(S_T)]
                        for s in range(S_T):
                            nc.sync.dma_start(
                                out=ETc[1][s][:],
                                in_=EnD[512:1024, s * P : (s + 1) * P],
                                transpose=True)

            # ---- C: y'T = x^T ETc per chunk; then zT = W2^T y'T + c ------
            # C1 loops s-outer with all 8 td-banks live, so the first MMs
            # need only ETc[0] and consumption trails the transposed loads
            # without stalling.
            with tc.tile_pool(name="pC", bufs=1) as pC:
                ycs = [[None] * D_T for _ in range(QC)]
                for qc in range(QC):
                    psC = [psp.tile([P, 512], F32, tag="ps", name="ps")
                           for _ in range(D_T)]
                    for s in range(S_T):
                        for td in range(D_T):
                            nc.tensor.matmul(
                                psC[td][:], xOs[s][:, td * P : (td + 1) * P],
                                ETc[qc][s][:],
                                start=(s == 0), stop=(s == S_T - 1))
                    for td in range(D_T):
                        yc = pC.tile([P, 512], F16, tag=f"yc{qc}_{td}",
                                     name=f"yc{qc}_{td}")
                        nc.scalar.activation(yc[:], psC[td][:], Ident)
                        ycs[qc][td] = yc
                for td in range(D_T):
                    ds_ = slice(td * P, (td + 1) * P)
                    psZ = [psp.tile([P, 512], F32, tag="ps", name="ps")
                           for _ in range(QC)]
                    for d in range(D_T):
                        for qc in range(QC):
                            nc.tensor.matmul(
                                psZ[qc][:], W2s[d][:, ds_], ycs[qc][d][:],
                                start=(d == 0), stop=(d == D_T - 1))
                    for qc in range(QC):
                        zsb = pC.tile([P, 512], F16, tag="zsb", name="zsb",
                                      bufs=2)
                        nc.scalar.activation(zsb[:], psZ[qc][:], Ident,
                                             bias=bias_c[:, td : td + 1])
                        nc.sync.dma_start(
                            out=zT[ds_, qc * 512 : (qc + 1) * 512],
                            in_=zsb[:])

    if split_waits:
        _split_multi_waits(nc)
    return nc


_NC = {}


def _get_nc():
    if "nc" not in _NC:
        _NC["nc"] = _build()
    return _NC["nc"]


def _in_maps(x, Wq, bq, Wk, bk, Wv, bv, Wo, bo):
    x = np.asarray(x, np.float32)
    Wq = np.asarray(Wq, np.float32)
    Wk = np.asarray(Wk, np.float32)
    Wv = np.asarray(Wv, np.float32)
    Wo = np.asarray(Wo, np.float32)
    M = (Wq @ Wk.T).astype(np.float16)
    W2 = (Wv @ Wo).astype(np.float16)
    r = (Wk @ np.asarray(bq, np.float32)).astype(np.float32)
    c = (Wo.T @ np.asarray(bv, np.float32) + np.asarray(bo, np.float32)).astype(
        np.float32)
    x16 = x.astype(np.float16)
    com = {"M": M, "W2": W2, "r": r, "c": c}
    maps = []
    for core in range(8):
        b, h = divmod(core, 2)
        xb = x16[b]                      # [S, D]
        if h:                            # rotate so queries sit at rows 0:NQ
            xb = np.concatenate([xb[NQ:], xb[:NQ]], axis=0)
        m = dict(com)
        m["xO"] = np.ascontiguousarray(xb)
        m["xT"] = np.ascontiguousarray(xb.T)
        maps.append(m)
    return maps


def kernel(x, Wq, bq, Wk, bk, Wv, bv, Wo, bo, _trace=False):
    nc = _get_nc()
    maps = _in_maps(x, Wq, bq, Wk, bk, Wv, bv, Wo, bo)
    res = run_bass_kernel_spmd(nc, maps, list(range(8)), trace=_trace)
    out = np.empty((B, S, D), np.float32)
    for core in range(8):
        b, h = divmod(core, 2)
        out[b, h * NQ : (h + 1) * NQ, :] = res.results[core]["zT"].T.astype(
            np.float32)
    if _trace:
        kernel.last_exec_time_ns = res.exec_time_ns
        kernel.last_profile = res
    return out


# revision 24
# speedup vs baseline: 1.0176x; 1.0176x over previous
"""Trainium2 Bass kernel for single-head attention (B=4, S=2048, D=H=1024).

Sharding: 8 cores = 4 batches x 2 query-halves. Each core computes the
attention output for 1024 query rows of one batch. The sequence axis is
rotated per-core on the host so the core's query rows always occupy
positions 0..1023 (softmax and the sum over keys are permutation-
invariant, so key order doesn't matter as long as xT/xO agree).

Algebraic restructuring (weights-only folding, done host-side in fp32):
  M  = Wq @ Wk^T          [D,D]   scores = x_q M x_k^T (+ r.x_k)
  r  = Wk @ bq            [D]     per-key score bias; the per-query terms
                                  (x_q Wq).bk + bq.bk are softmax-invariant
                                  and dropped
  W2 = Wv @ Wo            [D,D]   z = (En @ x) @ W2 + c
  c  = Wo^T @ bv + bo     [D]     (uses sum_k En = 1)
This removes the K and V projections entirely: the score and output GEMMs
contract directly against x. Per-core matmul work drops from 10 x 2^30 to
6 x 2^30 MACs (768 N=512 matmuls).

Per-core pipeline (all matmuls fp16 operands, fp32 PSUM):
  A1: TT[j,q] = M^T xq + r   (SBUF-resident, 128 MMs; stationary M-block
      reused across the two 512-query chunks)
  B:  per q-tile: S = TT^T xT (256 MMs; stationary TT-block reused across
      the four 512-key chunks) -> rowmax -> Exp(bias=-max, accum_out=den)
      -> En = E/den fp16 -> DRAM (EnD). ETc transpose-loads for each
      512-query chunk are issued as soon as its 4 q-tiles are stored.
  C:  y'T = x^T ETc per chunk (256 MMs), then zT = W2^T y'T + c for both
      chunks (128 MMs; stationary W2-block reused across chunks) -> DRAM.
      Host transposes zT back to [B,S,D].
"""

import sys

import numpy as np

for _p in ("/opt/trn_rl_repo",):
    if _p not in sys.path:
        sys.path.insert(0, _p)

import concourse.bass as bass
import concourse.mybir as mybir
import concourse.tile as tile
from concourse.bass_utils import run_bass_kernel_spmd


def _install_profile_shims():
    """This image's `antenv` lacks `axon_hooks`, which run_bass_kernel_spmd
    imports for trace=True under axon; libaxon_pjrt.so has the NTFF symbols.
    Register a stand-in module wired to the ctypes hook, and neuter the
    artifact upload (zero-egress container)."""
    import types

    try:
        import antenv.axon_hooks  # noqa: F401
    except ImportError:
        hook = None
        try:
            import trn_agent_boot.trn_boot as _tb

            hook = _tb._ntff_profile_via_ctypes("/opt/axon/libaxon_pjrt.so")
        except Exception:
            hook = None
        import antenv

        m = types.ModuleType("antenv.axon_hooks")
        m.get_axon_ntff_profile_hook = lambda: hook
        m.set_axon_ntff_profile_hook = lambda h: None
        sys.modules["antenv.axon_hooks"] = m
        antenv.axon_hooks = m

    import concourse.bass_utils as _bu

    _bu.upload_artifacts = lambda tmpdir: tmpdir


_install_profile_shims()

B, S, D, H = 4, 2048, 1024, 1024
P = 128
NQ = 1024  # query rows per core
D_T, S_T, Q_T = D // P, S // P, NQ // P
KC, QC = S // 512, NQ // 512

F32 = mybir.dt.float32
F16 = mybir.dt.float16
Ident = mybir.ActivationFunctionType.Identity


def _split_multi_waits(nc, max_waits=1):
    """This container's walrus rejects >1 sync wait on NO_STRUCT opcodes
    (Drain/NoOp). Move extra waits onto dedicated single-wait NoOps inserted
    right before the offending instruction on the same engine."""
    for f in nc.m.functions:
        for bb in f.blocks:
            insts = bb.instructions
            i = 0
            while i < len(insts):
                ins = insts[i]
                si = ins.sync_info
                if si is not None and si.on_wait and len(si.on_wait) > max_waits:
                    waits = list(si.on_wait)
                    si.on_wait = waits[:max_waits]
                    ins.sync_info = si
                    for j, w in enumerate(waits[max_waits:]):
                        nop = mybir.InstNoOp(
                            name=f"{ins.name}-waitsplit-{j}",
                            engine=ins.engine,
                            bass_nofuse=True,
                            sync_info=mybir.SyncInfo(on_wait=[w], on_update=[]),
                        )
                        insts.insert(i, nop)
                        i += 1
                i += 1
            bb.instructions = insts


def _build(split_waits=True):
    nc = bass.Bass()

    def din(name, shape, dt=F16):
        return nc.declare_dram_parameter(name, shape, dt, isOutput=False)

    xTd = din("xT", [D, S])      # x[b]^T, seq rotated so queries at cols 0:NQ
    xOd = din("xO", [S, D])      # x[b], same rotation on rows
    Md = din("M", [D, D])        # Wq Wk^T fp16
    W2d = din("W2", [D, D])      # Wv Wo   fp16
    rd = din("r", [D], F32)      # Wk bq
    cd = din("c", [D], F32)      # Wo^T bv + bo
    zT = nc.declare_dram_parameter("zT", [D, NQ], F16, isOutput=True)

    with tile.TileContext(nc) as tc:
        with (
            tc.tile_pool(name="pers", bufs=1) as pers,
            tc.tile_pool(name="dram", bufs=1, space="DRAM") as dramp,
            tc.tile_pool(name="ps", bufs=8, space="PSUM") as psp,
        ):
            EnD = dramp.tile([NQ, S], F16, tag="EnD", name="EnD")

            # --- DMA priority order: first MM group needs M[0] + xq(xT[0]).
            # xT tiles are loaded in two 1024-col halves so the query half
            # lands first. All loads stay on the sync HWDGE queue: moving
            # some to the scalar queue measurably delayed them (late queue
            # start + contention with Act compute).
            Ms = [pers.tile([P, D], F16, tag=f"m{d}", name=f"m{d}")
                  for d in range(D_T)]
            xTs = [pers.tile([P, S], F16, tag=f"xt{d}", name=f"xt{d}")
                   for d in range(D_T)]
            nc.sync.dma_start(out=Ms[0][:], in_=Md[0:P, :])
            nc.sync.dma_start(out=xTs[0][:, 0:NQ], in_=xTd[0:P, 0:NQ])
            for d in range(1, D_T):
                r_ = slice(d * P, (d + 1) * P)
                nc.sync.dma_start(out=Ms[d][:], in_=Md[r_, :])
                nc.sync.dma_start(out=xTs[d][:, 0:NQ], in_=xTd[r_, 0:NQ])
            bias_r = pers.tile([P, D_T], F32, tag="br", name="br")
            bias_c = pers.tile([P, D_T], F32, tag="bc", name="bc")
            nc.sync.dma_start(out=bias_r[:], in_=rd.rearrange("(t p) -> p t", p=P))
            nc.sync.dma_start(out=bias_c[:], in_=cd.rearrange("(t p) -> p t", p=P))
            for d in range(D_T):
                r_ = slice(d * P, (d + 1) * P)
                nc.sync.dma_start(out=xTs[d][:, NQ:S], in_=xTd[r_, NQ:S])
            xOs = []
            for s in range(S_T):
                r_ = slice(s * P, (s + 1) * P)
                ot = pers.tile([P, D], F16, tag=f"xo{s}", name=f"xo{s}")
                nc.sync.dma_start(out=ot[:], in_=xOd[r_, :])
                xOs.append(ot)
            W2s = []
            for d in range(D_T):
                r_ = slice(d * P, (d + 1) * P)
                wt = pers.tile([P, D], F16, tag=f"w2{d}", name=f"w2{d}")
                nc.sync.dma_start(out=wt[:], in_=W2d[r_, :])
                W2s.append(wt)

            # TT: [j, q] fp16, 8 j-tiles of [128, 1024]
            TT = [pers.tile([P, NQ], F16, tag=f"tt{t}", name=f"tt{t}")
                  for t in range(D_T)]

            # --- PE warmup: matmuls on a zeroed tile while input DMAs land,
            # so the HAM un-throttles before real work starts.
            warm = pers.tile([P, P], F16, tag="warm", name="warm")
            nc.vector.memset(warm[:], 0.0)
            wps = psp.tile([P, 512], F32, tag="ps", name="ps")
            for _ in range(32):
                nc.tensor.matmul(wps[:, 0:P], warm[:], warm[:],
                                 start=True, stop=True)

            # ---- A1: TT = M^T xq + r. d-outer; within d, the M-block
            # stationary operand is reused for both 512-query chunks.
            # t split into two halves so 2qc x 4t = 8 PSUM banks.
            for th in range(2):
                ts = range(th * 4, th * 4 + 4)
                psA = {(qc, t): psp.tile([P, 512], F32, tag="ps", name="ps")
                       for t in ts for qc in range(QC)}
                for d in range(D_T):
                    for t in ts:
                        for qc in range(QC):
                            nc.tensor.matmul(
                                psA[qc, t][:], Ms[d][:, t * P : (t + 1) * P],
                                xTs[d][:, qc * 512 : (qc + 1) * 512],
                                start=(d == 0), stop=(d == D_T - 1))
                for t in ts:
                    for qc in range(QC):
                        nc.scalar.activation(
                            TT[t][:, qc * 512 : (qc + 1) * 512],
                            psA[qc, t][:], Ident,
                            bias=bias_r[:, t : t + 1])

            # ---- B: scores + softmax -> EnD; ETc prefetch per 512-q chunk -
            ETc = [[None] * S_T for _ in range(QC)]
            with tc.tile_pool(name="pB", bufs=2) as pB:
                for qt in range(Q_T):
                    qsl = slice(qt * P, (qt + 1) * P)
                    Ssb = pB.tile([P, S], F32, tag="Ssb", name="Ssb")
                    pss = [psp.tile([P, 512], F32, tag="ps", name="ps")
                           for _ in range(KC)]
                    for t in range(D_T):
                        for kc in range(KC):
                            nc.tensor.matmul(
                                pss[kc][:], TT[t][:, qsl],
                                xTs[t][:, kc * 512 : (kc + 1) * 512],
                                start=(t == 0), stop=(t == D_T - 1))
                    for kc in range(KC):
                        nc.vector.tensor_copy(
                            Ssb[:, kc * 512 : (kc + 1) * 512], pss[kc][:])
                    nmx = pB.tile([P, 1], F32, tag="nmx", name="nmx")
                    nc.vector.reduce_max(nmx[:], Ssb[:],
                                         axis=mybir.AxisListType.X,
                                         negate=True)
                    En = pB.tile([P, S], F16, tag="En", name="En")
                    den = pB.tile([P, 1], F32, tag="den", name="den")
                    nc.scalar.activation(
                        En[:], Ssb[:], mybir.ActivationFunctionType.Exp,
                        bias=nmx[:], accum_out=den[:])
                    rec = pB.tile([P, 1], F32, tag="rec", name="rec")
                    nc.vector.reciprocal(rec[:], den[:])
                    Enn = pB.tile([P, S], F16, tag="Enn", name="Enn")
                    nc.scalar.mul(Enn[:], En[:], rec[:])
                    nc.sync.dma_start(out=EnD[qt * P : (qt + 1) * P, :],
                                      in_=Enn[:])
                    # Transposed ETc reads. One queue only: concurrent
                    # DMA_TRANSPOSE from two HWDGE queues corrupts data
                    # (shared XBAR), and finer-grained [128,128] transposes
                    # hang the exec unit. Chunk a's 16 triggers (~1.3us
                    # each) are spread in 4-batches across qt 3..6 so the
                    # qt4-7 Enn writes (which gate chunk b's transposes)
                    # are not starved behind a 20us trigger burst.
                    if qt == 3:
                        ETc[0] = [pers.tile([P, 512], F16, tag=f"et{s}",
                                            name=f"et{s}", bufs=2)
                                  for s in range(S_T)]
                    if 3 <= qt <= 6:
                        for s in range(4 * (qt - 3), 4 * (qt - 3) + 4):
                            nc.sync.dma_start(
                                out=ETc[0][s][:],
                                in_=EnD[0:512, s * P : (s + 1) * P],
                                transpose=True)
                    if qt == 7:
                        ETc[1] = [pers.tile([P, 512], F16, tag=f"et{s}",
                                            name=f"et{s}", bufs=2)
                                  for s in range(S_T)]
                        for s in range(S_T):
                            nc.sync.dma_start(
                                out=ETc[1][s][:],
                                in_=EnD[512:1024, s * P : (s + 1) * P],
                                transpose=True)

            # ---- C: y'T = x^T ETc per chunk; then zT = W2^T y'T + c ------
            # C1 loops s-outer with all 8 td-banks live, so the first MMs
            # need only ETc[0] and consumption trails the transposed loads
            # without stalling.
            with tc.tile_pool(name="pC", bufs=1) as pC:
                ycs = [[None] * D_T for _ in range(QC)]
                for qc in range(QC):
                    psC = [psp.tile([P, 512], F32, tag="ps", name="ps")
                           for _ in range(D_T)]
                    for s in range(S_T):
                        for td in range(D_T):
                            nc.tensor.matmul(
                                psC[td][:], xOs[s][:, td * P : (td + 1) * P],
                                ETc[qc][s][:],
                                start=(s == 0), stop=(s == S_T - 1))
                    for td in range(D_T):
                        yc = pC.tile([P, 512], F16, tag=f"yc{qc}_{td}",
                                     name=f"yc{qc}_{td}")
                        nc.scalar.activation(yc[:], psC[td][:], Ident)
                        ycs[qc][td] = yc
                for td in range(D_T):
                    ds_ = slice(td * P, (td + 1) * P)
                    psZ = [psp.tile([P, 512], F32, tag="ps", name="ps")
                           for _ in range(QC)]
                    for d in range(D_T):
                        for qc in range(QC):
                            nc.tensor.matmul(
                                psZ[qc][:], W2s[d][:, ds_], ycs[qc][d][:],
                                start=(d == 0), stop=(d == D_T - 1))
                    for qc in range(QC):
                        zsb = pC.tile([P, 512], F16, tag="zsb", name="zsb",
                                      bufs=2)
                        nc.scalar.activation(zsb[:], psZ[qc][:], Ident,
                                             bias=bias_c[:, td : td + 1])
                        nc.sync.dma_start(
                            out=zT[ds_, qc * 512 : (qc + 1) * 512],
                            in_=zsb[:])

    if split_waits:
        _split_multi_waits(nc)
    return nc


_NC = {}


def _get_nc():
    if "nc" not in _NC:
        _NC["nc"] = _build()
    return _NC["nc"]


def _in_maps(x, Wq, bq, Wk, bk, Wv, bv, Wo, bo):
    x = np.asarray(x, np.float32)
    Wq = np.asarray(Wq, np.float32)
    Wk = np.asarray(Wk, np.float32)
    Wv = np.asarray(Wv, np.float32)
    Wo = np.asarray(Wo, np.float32)
    M = (Wq @ Wk.T).astype(np.float16)
    W2 = (Wv @ Wo).astype(np.float16)
    r = (Wk @ np.asarray(bq, np.float32)).astype(np.float32)
    c = (Wo.T @ np.asarray(bv, np.float32) + np.asarray(bo, np.float32)).astype(
        np.float32)
    x16 = x.astype(np.float16)
    com = {"M": M, "W2": W2, "r": r, "c": c}
    maps = []
    for core in range(8):
        b, h = divmod(core, 2)
        xb = x16[b]                      # [S, D]
        if h:                            # rotate so queries sit at rows 0:NQ
            xb = np.concatenate([xb[NQ:], xb[:NQ]], axis=0)
        m = dict(com)
        m["xO"] = np.ascontiguousarray(xb)
        m["xT"] = np.ascontiguousarray(xb.T)
        maps.append(m)
    return maps


def kernel(x, Wq, bq, Wk, bk, Wv, bv, Wo, bo, _trace=False):
    nc = _get_nc()
    maps = _in_maps(x, Wq, bq, Wk, bk, Wv, bv, Wo, bo)
    res = run_bass_kernel_spmd(nc, maps, list(range(8)), trace=_trace)
    out = np.empty((B, S, D), np.float32)
    for core in range(8):
        b, h = divmod(core, 2)
        out[b, h * NQ : (h + 1) * NQ, :] = res.results[core]["zT"].T.astype(
            np.float32)
    if _trace:
        kernel.last_exec_time_ns = res.exec_time_ns
        kernel.last_profile = res
    return out
